# revision 1
# baseline (speedup 1.0000x reference)
"""FBPINN forward kernel for Trainium2 (8 NeuronCores), MoE-routing style.

Strategy
--------
The reference evaluates all S=64 subdomain MLPs densely on all N=131072
points, then combines with a sigmoid-product window w_s(x) normalized over
S.  The window decays like exp(-s_x * d) with s_x ~ 4266 beyond each
subdomain's core cell, so each point has non-negligible w for at most 2
subdomains.  We route points to subdomains on the host (exact interval
test: every dropped (s, point) pair has window sigmoid args <= -34, i.e.
w < 1.7e-15, far below fp32 resolution of the normalized sum), pad each
subdomain's point list to a common PAD, and evaluate the tiny MLPs on
device, expert-parallel: 8 subdomains per core, packed 4-at-a-time into
block-diagonal 128-wide matmuls.

Device does: x -> [block-diag in-proj; input normalization and bias are
folded into the weights via a ones row] -> tanh -> 2x [block-diag 32x32
hidden + per-partition bias] -> tanh -> block-diag out-proj (fp32
matmuls throughout: float32r/bf16 were measured 2.8e-3/1e-2 rel err,
too lossy vs the fp32 reference).
Host does: routing, window weights, scatter-add normalization, boundary
condition. Cross-subdomain reduction happens in the host scatter-add, so
no collectives are needed.
"""

import numpy as np
from contextlib import ExitStack

S = 64
N_DIM = 2
H = 32
SCALE, SHIFT = 1.0, 0.0
NCORES = 8
SUB_PER_CORE = S // NCORES      # 8
G = 2                           # groups of 4 subdomains per core
TAU = 12.0                      # dropped window weight <~1e-5 of scale; measured vs fp64 oracle
T = 512                         # device column tile

_BUILD_CACHE = {}


def _build_bass(pad):
    import concourse.bass as bass
    import concourse.tile as tile
    from concourse import bacc, mybir

    f32 = mybir.dt.float32
    nc = bacc.Bacc("TRN2", target_bir_lowering=False, debug=False,
                   num_devices=NCORES)
    xb = nc.dram_tensor("xb", [G, 9, pad], f32, kind="ExternalInput").ap()
    wb = nc.dram_tensor("wb", [G, 128, 390], f32, kind="ExternalInput").ap()
    o = nc.dram_tensor("o", [G, 4, pad], f32, kind="ExternalOutput").ap()

    tanh = mybir.ActivationFunctionType.Tanh

    with tile.TileContext(nc) as tc, ExitStack() as ctx:
        consts = ctx.enter_context(tc.tile_pool(name="consts", bufs=1))
        hpool = ctx.enter_context(tc.tile_pool(name="hs", bufs=3))
        opool = ctx.enter_context(tc.tile_pool(name="os", bufs=1))
        psum = ctx.enter_context(tc.tile_pool(name="ps", bufs=2, space="PSUM"))

        # One weight-blob DMA + one xb DMA per group: 6 DMAs total stay
        # within the 8 HWDGE queues, so no DMA ever carries a queue-reuse
        # wait on top of its data wait (1-wait budget per instruction).
        xb_t, wb_t, wi_t, wh_t, bh_t, wo_t, o_sb = {}, {}, {}, {}, {}, {}, {}
        for g in range(G):
            xb_t[g] = consts.tile([9, pad], f32, tag=f"xb{g}", name=f"xbt{g}")
            nc.sync.dma_start(out=xb_t[g][:], in_=xb[g])
            wb_t[g] = consts.tile([128, 390], f32, tag=f"wb{g}", name=f"wbt{g}")
            nc.sync.dma_start(out=wb_t[g][:], in_=wb[g])
            wi_t[g] = wb_t[g][0:9, 0:128]
            wh_t[g, 0] = wb_t[g][:, 128:256]
            wh_t[g, 1] = wb_t[g][:, 256:384]
            wo_t[g] = wb_t[g][:, 384:388]
            bh_t[g, 0] = wb_t[g][:, 388:389]
            bh_t[g, 1] = wb_t[g][:, 389:390]

        # Throwaway accumulation-group matmuls absorb the preamble DMA
        # semaphore waits into the PE clock, so steady-state matmuls carry
        # at most one wait each.
        dp = psum.tile([1, 1], f32, tag="dp", bufs=1, name="dp")

        # Warm the PE_HAM clock gate (1.2 -> 2.4 GHz needs ~3.4 us of
        # sustained PE activity) while the input DMAs are still in flight:
        # stream a memset tile through the array a few times.
        warm = hpool.tile([128, T], f32, tag="warm", name="warm")
        nc.vector.memset(warm[:], 0.0)
        wp = psum.tile([1, T], f32, tag="dp", bufs=1, name="wp",
                       padded_shape=[1, T])
        for i in range(2):
            nc.tensor.matmul(wp[:], warm[:, 0:1], warm[:],
                             start=True, stop=True, skip_group_check=True)

        sizes = [T] * (pad // T)
        if pad % T:
            sizes.append(pad % T)
        # split the final tile so the last iteration's serial
        # p1->tanh->...->po chain (which nothing overlaps) is short
        if sizes[-1] > 256:
            sizes[-1:] = [sizes[-1] - 128, 128]
        offs = [sum(sizes[:i]) for i in range(len(sizes))]
        nbufs = G * len(sizes)      # unique slot per iteration: no SBUF WAW
        for g in range(G):
            # absorb this group's DMA-queue waits just before its loop, so
            # group 0 compute is not gated on group 1's DMAs (PE is in-order)
            for i, wt in enumerate((wb_t[g], xb_t[g])):
                w1 = wt[:, 0:1].bitcast(f32)
                nc.tensor.matmul(dp[:], w1, w1, start=(g == 0 and i == 0),
                                 stop=(g == G - 1 and i == 1),
                                 skip_group_check=True)
            o_sb[g] = opool.tile([4, pad], f32, tag=f"o{g}", name=f"osb{g}")
            for it, (off, tsz) in enumerate(zip(offs, sizes)):
                rhs = xb_t[g][:, off:off + tsz]
                p1 = psum.tile([128, tsz], f32, tag="p1",
                               padded_shape=[128, T])
                nc.tensor.matmul(p1[:], wi_t[g], rhs, start=True, stop=True)
                h1 = hpool.tile([128, tsz], f32, tag="h1", bufs=nbufs,
                                padded_shape=[128, T])
                nc.scalar.activation(h1[:], p1[:], tanh)
                p2 = psum.tile([128, tsz], f32, tag="p2",
                               padded_shape=[128, T])
                nc.tensor.matmul(p2[:], wh_t[g, 0], h1[:], start=True, stop=True)
                h2 = hpool.tile([128, tsz], f32, tag="h2", bufs=nbufs,
                                padded_shape=[128, T])
                nc.scalar.activation(h2[:], p2[:], tanh, bias=bh_t[g, 0])
                p3 = psum.tile([128, tsz], f32, tag="p3",
                               padded_shape=[128, T])
                nc.tensor.matmul(p3[:], wh_t[g, 1], h2[:], start=True, stop=True)
                h3 = hpool.tile([128, tsz], f32, tag="h3", bufs=nbufs,
                                padded_shape=[128, T])
                nc.scalar.activation(h3[:], p3[:], tanh, bias=bh_t[g, 1])
                po = psum.tile([4, tsz], f32, tag="po", bufs=1,
                               padded_shape=[4, T])
                nc.tensor.matmul(po[:], wo_t[g], h3[:], start=True, stop=True)
                nc.vector.tensor_copy(o_sb[g][:, off:off + tsz], po[:])
            nc.sync.dma_start(out=o[g], in_=o_sb[g][:])
    nc.compile()
    return nc


def _route(x, lo_core, hi_core, swin):
    """Per-subdomain point lists: s covers p iff all window sigmoid args >= -TAU."""
    n = x.shape[0]
    pts = []
    for si in range(S):
        m = np.ones(n, dtype=bool)
        for d in range(N_DIM):
            sd = swin[si, d]
            lo, hi = lo_core[si, d], hi_core[si, d]
            if sd >= 0:
                m &= (x[:, d] >= lo - TAU / max(sd, 1e-30)) \
                    & (x[:, d] <= hi + TAU / max(sd, 1e-30))
            else:  # pathological geometry; sigmoids flip direction
                m &= (x[:, d] <= lo + TAU / max(-sd, 1e-30)) \
                    & (x[:, d] >= hi - TAU / max(-sd, 1e-30))
        pts.append(np.nonzero(m)[0])
    return pts


def _pack(x, args64, pts, pad, Wn, bn):
    """Build the per-core device input tensors."""
    in_maps = []
    for c in range(NCORES):
        xb = np.zeros((G, 9, pad), np.float32)
        wbv = np.zeros((G, 128, 390), np.float32)
        wi = wbv[:, 0:9, 0:128]
        wh0 = wbv[:, :, 128:256]
        wh1 = wbv[:, :, 256:384]
        wo = wbv[:, :, 384:388]
        bh0 = wbv[:, :, 388]
        bh1 = wbv[:, :, 389]
        for g in range(G):
            for j in range(4):
                s_ = c * SUB_PER_CORE + g * 4 + j
                idx = pts[s_]
                cnt = len(idx)
                xs = x[idx]
                xb[g, 0, :] = 1.0
                xb[g, 1 + 2 * j, :cnt] = xs[:, 0]
                xb[g, 2 + 2 * j, :cnt] = xs[:, 1]
                r = slice(32 * j, 32 * j + 32)
                for d in range(N_DIM):
                    wi[g, 1 + 2 * j + d, r] = Wn[s_, :, d]
                wi[g, 0, r] = bn[s_]
                wh0[g, r, r] = args64["W_h1"][s_].T
                wh1[g, r, r] = args64["W_h2"][s_].T
                bh0[g, r] = args64["b_h1"][s_]
                bh1[g, r] = args64["b_h2"][s_]
                wo[g, r, j] = args64["W_out"][s_, 0]
        in_maps.append({"xb": xb, "wb": wbv})
    return in_maps


def _host_reference(x, lo_core, hi_core, lo_ext, hi_ext,
                    W_in, b_in, W_h1, b_h1, W_h2, b_h2, W_out, b_out):
    """Dense fallback (numpy, chunked) for inputs without FBPINN locality."""
    center = (lo_ext + hi_ext) * 0.5
    half_w = (hi_ext - lo_ext) * 0.5
    overlap = np.maximum(hi_ext - hi_core, lo_core - lo_ext)
    width = hi_ext - lo_ext
    s = 4.0 / (2.0 * overlap * width + 1e-8)
    sigm = lambda v: 1.0 / (1.0 + np.exp(-v))
    outs = []
    for i in range(0, x.shape[0], 8192):
        xc = x[i:i + 8192].astype(np.float64)
        xn = (xc[None] - center[:, None]) / half_w[:, None]
        hh = np.tanh(np.einsum("snd,shd->snh", xn, W_in) + b_in[:, None])
        hh = np.tanh(np.einsum("snh,skh->snk", hh, W_h1) + b_h1[:, None])
        hh = np.tanh(np.einsum("snh,skh->snk", hh, W_h2) + b_h2[:, None])
        out = np.einsum("snh,soh->sno", hh, W_out) + b_out[:, None]
        out = out * SCALE + SHIFT
        left = sigm(s[:, None] * (xc[None] - lo_core[:, None]))
        right = sigm(s[:, None] * (hi_core[:, None] - xc[None]))
        w = np.prod(left * right, axis=-1, keepdims=True)
        w = w / (np.sum(w, axis=0, keepdims=True) + 1e-8)
        u = np.sum(out * w, axis=0)
        gg = -np.sin(np.pi * xc[:, 1])[:, None]
        fac = (np.tanh(xc[:, 1] + 1) * np.tanh(xc[:, 1] - 1)
               * np.tanh(xc[:, 0]))[:, None]
        outs.append((gg + fac * u).astype(np.float32))
    return np.concatenate(outs, axis=0)


def _prepare(x, args64):
    """Routing + weight folding. Returns (pts, pad, swin, Wn, bn) or None
    if the inputs lack FBPINN locality (caller should fall back to dense)."""
    lo_core64, hi_core64 = args64["lo_core"], args64["hi_core"]
    lo_ext64, hi_ext64 = args64["lo_ext"], args64["hi_ext"]
    n = x.shape[0]
    center = (lo_ext64 + hi_ext64) * 0.5
    half_w = (hi_ext64 - lo_ext64) * 0.5
    overlap = np.maximum(hi_ext64 - hi_core64, lo_core64 - lo_ext64)
    width = hi_ext64 - lo_ext64
    swin = 4.0 / (2.0 * overlap * width + 1e-8)

    pts = _route(x, lo_core64, hi_core64, swin)
    counts = np.array([len(p) for p in pts])
    if counts.sum() > 4 * n or counts.max() > max(4 * n // S, 8192):
        return None
    pad = int(max(128, -(-counts.max() // 128) * 128))

    W_in64 = args64["W_in"]                      # (S,H,D)
    Wn = W_in64 / half_w[:, None, :]             # (S,H,D)
    bn = args64["b_in"] - np.einsum("shd,sd->sh", W_in64, center / half_w)
    return pts, pad, swin, Wn, bn


def _epilogue(x, args64, pts, swin, o_by_sub):
    """Window weights + normalized scatter-add + boundary condition.
    o_by_sub: callable s -> raw device MLP outputs for subdomain s's slots."""
    n = x.shape[0]
    lo_core64, hi_core64 = args64["lo_core"], args64["hi_core"]
    b_out64 = args64["b_out"]
    numer = np.zeros(n, np.float64)
    denom = np.zeros(n, np.float64)
    sigm = lambda v: 1.0 / (1.0 + np.exp(-v))
    for s_ in range(S):
        idx = pts[s_]
        cnt = len(idx)
        if cnt == 0:
            continue
        xs = x[idx].astype(np.float64)
        arg_l = swin[s_] * (xs - lo_core64[s_])
        arg_r = swin[s_] * (hi_core64[s_] - xs)
        w = np.prod(sigm(arg_l) * sigm(arg_r), axis=-1)
        out_s = (o_by_sub(s_)[:cnt].astype(np.float64)
                 + b_out64[s_, 0]) * SCALE + SHIFT
        np.add.at(numer, idx, out_s * w)
        np.add.at(denom, idx, w)
    u = numer / (denom + 1e-8)
    x64 = x.astype(np.float64)
    gg = -np.sin(np.pi * x64[:, 1])
    fac = np.tanh(x64[:, 1] + 1.0) * np.tanh(x64[:, 1] - 1.0) * np.tanh(x64[:, 0])
    return (gg + fac * u)[:, None].astype(np.float32)


def kernel(x, lo_core, hi_core, lo_ext, hi_ext,
           W_in, b_in, W_h1, b_h1, W_h2, b_h2, W_out, b_out,
           _profile=False):
    x = np.asarray(x, np.float32)
    args64 = {k: np.asarray(v, np.float64) for k, v in dict(
        lo_core=lo_core, hi_core=hi_core, lo_ext=lo_ext, hi_ext=hi_ext,
        W_in=W_in, b_in=b_in, W_h1=W_h1, b_h1=b_h1, W_h2=W_h2, b_h2=b_h2,
        W_out=W_out, b_out=b_out).items()}

    prep = _prepare(x, args64)
    if prep is None:
        return _host_reference(x, **args64)
    pts, pad, swin, Wn, bn = prep

    in_maps = _pack(x, args64, pts, pad, Wn, bn)

    from concourse.bass_utils import run_bass_kernel_spmd
    if pad not in _BUILD_CACHE:
        _BUILD_CACHE[pad] = _build_bass(pad)
    nc = _BUILD_CACHE[pad]
    res = run_bass_kernel_spmd(nc, in_maps, list(range(NCORES)),
                               trace=bool(_profile))

    def o_by_sub(s_):
        c, rem = divmod(s_, SUB_PER_CORE)
        g, j = divmod(rem, 4)
        return res.results[c]["o"][g, j]

    final = _epilogue(x, args64, pts, swin, o_by_sub)
    if _profile:
        return final, res
    return final



# revision 9
# speedup vs baseline: 1.2644x; 1.2644x over previous
"""FBPINN forward kernel for Trainium2 (8 NeuronCores), MoE-routing style.

Strategy
--------
The reference evaluates all S=64 subdomain MLPs densely on all N=131072
points, then combines with a sigmoid-product window w_s(x) normalized over
S.  The window decays like exp(-s_x * d) with s_x ~ 4266 beyond each
subdomain's core cell, so each point has non-negligible w for at most 2
subdomains.  We route points to subdomains on the host (exact interval
test: every dropped (s, point) pair has window sigmoid args <= -34, i.e.
w < 1.7e-15, far below fp32 resolution of the normalized sum), pad each
subdomain's point list to a common PAD, and evaluate the tiny MLPs on
device, expert-parallel: 8 subdomains per core, packed 4-at-a-time into
block-diagonal 128-wide matmuls.

Device does: x -> [block-diag in-proj; input normalization and bias are
folded into the weights via a ones row] -> tanh -> 2x [block-diag 32x32
hidden + per-partition bias] -> tanh -> block-diag out-proj (fp32
matmuls throughout: float32r/bf16 were measured 2.8e-3/1e-2 rel err,
too lossy vs the fp32 reference).
Host does: routing, window weights, scatter-add normalization, boundary
condition. Cross-subdomain reduction happens in the host scatter-add, so
no collectives are needed.
"""

import numpy as np
from contextlib import ExitStack

S = 64
N_DIM = 2
H = 32
SCALE, SHIFT = 1.0, 0.0
NCORES = 8
SUB_PER_CORE = S // NCORES      # 8
G = 2                           # groups of 4 subdomains per core
TAU = 12.0                      # dropped window weight <~1e-5 of scale; measured vs fp64 oracle
T = 512                         # device column tile

_BUILD_CACHE = {}


def _build_bass(pad):
    import concourse.bass as bass
    import concourse.tile as tile
    from concourse import bacc, mybir

    f32 = mybir.dt.float32
    f32r = mybir.dt.float32r
    nc = bacc.Bacc("TRN2", target_bir_lowering=False, debug=False,
                   num_devices=NCORES)
    xb = nc.dram_tensor("xb", [G, 9, pad], f32r, kind="ExternalInput").ap()
    wb = nc.dram_tensor("wb", [G, 128, 390], f32r, kind="ExternalInput").ap()
    o = nc.dram_tensor("o", [G, 4, pad], f32, kind="ExternalOutput").ap()

    tanh = mybir.ActivationFunctionType.Tanh

    with tile.TileContext(nc) as tc, ExitStack() as ctx:
        consts = ctx.enter_context(tc.tile_pool(name="consts", bufs=1))
        hpool = ctx.enter_context(tc.tile_pool(name="hs", bufs=3))
        opool = ctx.enter_context(tc.tile_pool(name="os", bufs=1))
        psum = ctx.enter_context(tc.tile_pool(name="ps", bufs=2, space="PSUM"))

        # One weight-blob DMA + one xb DMA per group: 6 DMAs total stay
        # within the 8 HWDGE queues, so no DMA ever carries a queue-reuse
        # wait on top of its data wait (1-wait budget per instruction).
        xb_t, wb_t, wi_t, wh_t, bh_t, wo_t, o_sb = {}, {}, {}, {}, {}, {}, {}
        for g in range(G):
            xb_t[g] = consts.tile([9, pad], f32r, tag=f"xb{g}", name=f"xbt{g}")
            nc.sync.dma_start(out=xb_t[g][:], in_=xb[g])
            wb_t[g] = consts.tile([128, 390], f32r, tag=f"wb{g}", name=f"wbt{g}")
            nc.sync.dma_start(out=wb_t[g][:], in_=wb[g])
            wi_t[g] = wb_t[g][0:9, 0:128]
            wh_t[g, 0] = wb_t[g][:, 128:256]
            wh_t[g, 1] = wb_t[g][:, 256:384]
            wo_t[g] = wb_t[g][:, 384:388]
            bh_t[g, 0] = wb_t[g][:, 388:389].bitcast(f32)
            bh_t[g, 1] = wb_t[g][:, 389:390].bitcast(f32)

        # Throwaway accumulation-group matmuls absorb the preamble DMA
        # semaphore waits into the PE clock, so steady-state matmuls carry
        # at most one wait each.
        dp = psum.tile([1, 1], f32, tag="dp", bufs=1, name="dp")

        # Warm the PE_HAM clock gate (1.2 -> 2.4 GHz needs ~3.4 us of
        # sustained PE activity) while the input DMAs are still in flight:
        # stream a memset tile through the array a few times.
        warm = hpool.tile([128, T], f32, tag="warm", name="warm")
        nc.vector.memset(warm[:], 0.0)
        wp = psum.tile([1, T], f32, tag="dp", bufs=1, name="wp",
                       padded_shape=[1, T])
        for i in range(2):
            nc.tensor.matmul(wp[:], warm[:, 0:1], warm[:],
                             start=True, stop=True, skip_group_check=True)

        sizes = [T] * (pad // T)
        if pad % T:
            sizes.append(pad % T)
        # split the final tile so the last iteration's serial
        # p1->tanh->...->po chain (which nothing overlaps) is short
        if sizes[-1] > 256:
            sizes[-1:] = [sizes[-1] - 128, 128]
        offs = [sum(sizes[:i]) for i in range(len(sizes))]
        nbufs = G * len(sizes)      # unique slot per iteration: no SBUF WAW
        for g in range(G):
            # absorb this group's DMA-queue waits just before its loop, so
            # group 0 compute is not gated on group 1's DMAs (PE is in-order)
            for i, wt in enumerate((wb_t[g], xb_t[g])):
                w1 = wt[:, 0:1].bitcast(f32)
                nc.tensor.matmul(dp[:], w1, w1, start=(g == 0 and i == 0),
                                 stop=(g == G - 1 and i == 1),
                                 skip_group_check=True)
            o_sb[g] = opool.tile([4, pad], f32, tag=f"o{g}", name=f"osb{g}")
            for it, (off, tsz) in enumerate(zip(offs, sizes)):
                rhs = xb_t[g][:, off:off + tsz]
                p1 = psum.tile([128, tsz], f32, tag="p1",
                               padded_shape=[128, T])
                nc.tensor.matmul(p1[:], wi_t[g], rhs, start=True, stop=True)
                h1 = hpool.tile([128, tsz], f32r, tag="h1", bufs=nbufs,
                                padded_shape=[128, T])
                nc.scalar.activation(h1[:], p1[:], tanh)
                p2 = psum.tile([128, tsz], f32, tag="p2",
                               padded_shape=[128, T])
                nc.tensor.matmul(p2[:], wh_t[g, 0], h1[:], start=True, stop=True)
                h2 = hpool.tile([128, tsz], f32r, tag="h2", bufs=nbufs,
                                padded_shape=[128, T])
                nc.scalar.activation(h2[:], p2[:], tanh, bias=bh_t[g, 0])
                p3 = psum.tile([128, tsz], f32, tag="p3",
                               padded_shape=[128, T])
                nc.tensor.matmul(p3[:], wh_t[g, 1], h2[:], start=True, stop=True)
                h3 = hpool.tile([128, tsz], f32r, tag="h3", bufs=nbufs,
                                padded_shape=[128, T])
                nc.scalar.activation(h3[:], p3[:], tanh, bias=bh_t[g, 1])
                po = psum.tile([4, tsz], f32, tag="po", bufs=1,
                               padded_shape=[4, T])
                nc.tensor.matmul(po[:], wo_t[g], h3[:], start=True, stop=True)
                nc.vector.tensor_copy(o_sb[g][:, off:off + tsz], po[:])
            nc.sync.dma_start(out=o[g], in_=o_sb[g][:])
    nc.compile()
    return nc


def _route(x, lo_core, hi_core, swin):
    """Per-subdomain point lists: s covers p iff all window sigmoid args >= -TAU."""
    n = x.shape[0]
    pts = []
    for si in range(S):
        m = np.ones(n, dtype=bool)
        for d in range(N_DIM):
            sd = swin[si, d]
            lo, hi = lo_core[si, d], hi_core[si, d]
            if sd >= 0:
                m &= (x[:, d] >= lo - TAU / max(sd, 1e-30)) \
                    & (x[:, d] <= hi + TAU / max(sd, 1e-30))
            else:  # pathological geometry; sigmoids flip direction
                m &= (x[:, d] <= lo + TAU / max(-sd, 1e-30)) \
                    & (x[:, d] >= hi - TAU / max(-sd, 1e-30))
        pts.append(np.nonzero(m)[0])
    return pts


def _pack(x, args64, pts, pad, Wn, bn):
    """Build the per-core device input tensors."""
    in_maps = []
    for c in range(NCORES):
        xb = np.zeros((G, 9, pad), np.float32)
        wbv = np.zeros((G, 128, 390), np.float32)
        wi = wbv[:, 0:9, 0:128]
        wh0 = wbv[:, :, 128:256]
        wh1 = wbv[:, :, 256:384]
        wo = wbv[:, :, 384:388]
        bh0 = wbv[:, :, 388]
        bh1 = wbv[:, :, 389]
        for g in range(G):
            for j in range(4):
                s_ = c * SUB_PER_CORE + g * 4 + j
                idx = pts[s_]
                cnt = len(idx)
                xs = x[idx]
                xb[g, 0, :] = 1.0
                xb[g, 1 + 2 * j, :cnt] = xs[:, 0]
                xb[g, 2 + 2 * j, :cnt] = xs[:, 1]
                r = slice(32 * j, 32 * j + 32)
                for d in range(N_DIM):
                    wi[g, 1 + 2 * j + d, r] = Wn[s_, :, d]
                wi[g, 0, r] = bn[s_]
                wh0[g, r, r] = args64["W_h1"][s_].T
                wh1[g, r, r] = args64["W_h2"][s_].T
                bh0[g, r] = args64["b_h1"][s_]
                bh1[g, r] = args64["b_h2"][s_]
                wo[g, r, j] = args64["W_out"][s_, 0]
        in_maps.append({"xb": xb, "wb": wbv})
    return in_maps


def _host_reference(x, lo_core, hi_core, lo_ext, hi_ext,
                    W_in, b_in, W_h1, b_h1, W_h2, b_h2, W_out, b_out):
    """Dense fallback (numpy, chunked) for inputs without FBPINN locality."""
    center = (lo_ext + hi_ext) * 0.5
    half_w = (hi_ext - lo_ext) * 0.5
    overlap = np.maximum(hi_ext - hi_core, lo_core - lo_ext)
    width = hi_ext - lo_ext
    s = 4.0 / (2.0 * overlap * width + 1e-8)
    sigm = lambda v: 1.0 / (1.0 + np.exp(-v))
    outs = []
    for i in range(0, x.shape[0], 8192):
        xc = x[i:i + 8192].astype(np.float64)
        xn = (xc[None] - center[:, None]) / half_w[:, None]
        hh = np.tanh(np.einsum("snd,shd->snh", xn, W_in) + b_in[:, None])
        hh = np.tanh(np.einsum("snh,skh->snk", hh, W_h1) + b_h1[:, None])
        hh = np.tanh(np.einsum("snh,skh->snk", hh, W_h2) + b_h2[:, None])
        out = np.einsum("snh,soh->sno", hh, W_out) + b_out[:, None]
        out = out * SCALE + SHIFT
        left = sigm(s[:, None] * (xc[None] - lo_core[:, None]))
        right = sigm(s[:, None] * (hi_core[:, None] - xc[None]))
        w = np.prod(left * right, axis=-1, keepdims=True)
        w = w / (np.sum(w, axis=0, keepdims=True) + 1e-8)
        u = np.sum(out * w, axis=0)
        gg = -np.sin(np.pi * xc[:, 1])[:, None]
        fac = (np.tanh(xc[:, 1] + 1) * np.tanh(xc[:, 1] - 1)
               * np.tanh(xc[:, 0]))[:, None]
        outs.append((gg + fac * u).astype(np.float32))
    return np.concatenate(outs, axis=0)


def _prepare(x, args64):
    """Routing + weight folding. Returns (pts, pad, swin, Wn, bn) or None
    if the inputs lack FBPINN locality (caller should fall back to dense)."""
    lo_core64, hi_core64 = args64["lo_core"], args64["hi_core"]
    lo_ext64, hi_ext64 = args64["lo_ext"], args64["hi_ext"]
    n = x.shape[0]
    center = (lo_ext64 + hi_ext64) * 0.5
    half_w = (hi_ext64 - lo_ext64) * 0.5
    overlap = np.maximum(hi_ext64 - hi_core64, lo_core64 - lo_ext64)
    width = hi_ext64 - lo_ext64
    swin = 4.0 / (2.0 * overlap * width + 1e-8)

    pts = _route(x, lo_core64, hi_core64, swin)
    counts = np.array([len(p) for p in pts])
    if counts.sum() > 4 * n or counts.max() > max(4 * n // S, 8192):
        return None
    pad = int(max(128, -(-counts.max() // 128) * 128))

    W_in64 = args64["W_in"]                      # (S,H,D)
    Wn = W_in64 / half_w[:, None, :]             # (S,H,D)
    bn = args64["b_in"] - np.einsum("shd,sd->sh", W_in64, center / half_w)
    return pts, pad, swin, Wn, bn


def _epilogue(x, args64, pts, swin, o_by_sub):
    """Window weights + normalized scatter-add + boundary condition.
    o_by_sub: callable s -> raw device MLP outputs for subdomain s's slots."""
    n = x.shape[0]
    lo_core64, hi_core64 = args64["lo_core"], args64["hi_core"]
    b_out64 = args64["b_out"]
    numer = np.zeros(n, np.float64)
    denom = np.zeros(n, np.float64)
    sigm = lambda v: 1.0 / (1.0 + np.exp(-v))
    for s_ in range(S):
        idx = pts[s_]
        cnt = len(idx)
        if cnt == 0:
            continue
        xs = x[idx].astype(np.float64)
        arg_l = swin[s_] * (xs - lo_core64[s_])
        arg_r = swin[s_] * (hi_core64[s_] - xs)
        w = np.prod(sigm(arg_l) * sigm(arg_r), axis=-1)
        out_s = (o_by_sub(s_)[:cnt].astype(np.float64)
                 + b_out64[s_, 0]) * SCALE + SHIFT
        np.add.at(numer, idx, out_s * w)
        np.add.at(denom, idx, w)
    u = numer / (denom + 1e-8)
    x64 = x.astype(np.float64)
    gg = -np.sin(np.pi * x64[:, 1])
    fac = np.tanh(x64[:, 1] + 1.0) * np.tanh(x64[:, 1] - 1.0) * np.tanh(x64[:, 0])
    return (gg + fac * u)[:, None].astype(np.float32)


def kernel(x, lo_core, hi_core, lo_ext, hi_ext,
           W_in, b_in, W_h1, b_h1, W_h2, b_h2, W_out, b_out,
           _profile=False):
    x = np.asarray(x, np.float32)
    args64 = {k: np.asarray(v, np.float64) for k, v in dict(
        lo_core=lo_core, hi_core=hi_core, lo_ext=lo_ext, hi_ext=hi_ext,
        W_in=W_in, b_in=b_in, W_h1=W_h1, b_h1=b_h1, W_h2=W_h2, b_h2=b_h2,
        W_out=W_out, b_out=b_out).items()}

    prep = _prepare(x, args64)
    if prep is None:
        return _host_reference(x, **args64)
    pts, pad, swin, Wn, bn = prep

    in_maps = _pack(x, args64, pts, pad, Wn, bn)

    from concourse.bass_utils import run_bass_kernel_spmd
    if pad not in _BUILD_CACHE:
        _BUILD_CACHE[pad] = _build_bass(pad)
    nc = _BUILD_CACHE[pad]
    res = run_bass_kernel_spmd(nc, in_maps, list(range(NCORES)),
                               trace=bool(_profile))

    def o_by_sub(s_):
        c, rem = divmod(s_, SUB_PER_CORE)
        g, j = divmod(rem, 4)
        return res.results[c]["o"][g, j]

    final = _epilogue(x, args64, pts, swin, o_by_sub)
    if _profile:
        return final, res
    return final



# revision 12
# speedup vs baseline: 1.5371x; 1.2157x over previous
"""FBPINN forward kernel for Trainium2 (8 NeuronCores), MoE-routing style.

Strategy
--------
The reference evaluates all S=64 subdomain MLPs densely on all N=131072
points, then combines with a sigmoid-product window w_s(x) normalized over
S.  The window decays like exp(-s_x * d) with s_x ~ 4266 beyond each
subdomain's core cell, so each point has non-negligible w for at most 2
subdomains.  We route points to subdomains on the host (interval test:
every dropped (s, point) pair has window sigmoid args <= -TAU), pad each
subdomain's point list to a common PAD, and evaluate the tiny MLPs on
device, expert-parallel: 8 subdomains per core, packed 4-at-a-time into
block-diagonal 128-wide matmuls.

Device numerics: all matmuls bf16 (1 cyc/row on the PE vs 4 for fp32 and
2 for fp32r, and bf16 weight loads get FWL).  The in-projection carries
the per-subnet normalized coordinates as hi/lo bf16 pairs (two rows per
coordinate, same weight on both rows), which recovers ~2^-17 coordinate
precision for free: matmul cost scales with moving columns, not contract
rows.  PSUM accumulation stays fp32, tanh (ScalarE) reads fp32 PSUM over
1024-wide two-bank blocks and writes bf16 h tiles.  Measured end-to-end
rel err ~3e-3 vs the fp32 reference (gate 2e-2).
Host does: routing, window weights, scatter-add normalization, boundary
condition. Cross-subdomain reduction happens in the host scatter-add, so
no collectives are needed.
"""

import numpy as np
from contextlib import ExitStack

S = 64
N_DIM = 2
H = 32
SCALE, SHIFT = 1.0, 0.0
NCORES = 8
SUB_PER_CORE = S // NCORES      # 8
G = 2                           # groups of 4 subdomains per core
TAU = 9.0                       # dropped window weight <= e^-9 ~ 1.2e-4 relative
T = 512                         # matmul moving tile (one fp32 PSUM bank)
B = 1024                        # ACT block (two PSUM banks)
WCOLS = 388                     # wi 128 + wh0 128 + wh1 128 + wo 4

_BUILD_CACHE = {}


def _block_sizes(pad, last_group):
    """Split pad into <=B blocks; shorten the final serial chain of the
    program by splitting the last group's tail block."""
    sizes = [B] * (pad // B)
    if pad % B:
        sizes.append(pad % B)
    if last_group and sizes[-1] > 256:
        sizes[-1:] = [sizes[-1] - 128, 128]
    return sizes


def _mm_splits(bsz):
    out = []
    off = 0
    while off < bsz:
        m = min(T, bsz - off)
        out.append((off, m))
        off += m
    return out


def _build_bass(pad, has_bin, has_bh):
    import concourse.bass as bass
    import concourse.tile as tile
    from concourse import bacc, mybir

    f32 = mybir.dt.float32
    bf16 = mybir.dt.bfloat16
    R = 18 if has_bin else 16   # xb rows: (hi,lo)x2 coords x4 subnets [+2 ones]
    nc = bacc.Bacc("TRN2", target_bir_lowering=False, debug=False,
                   num_devices=NCORES)
    xb = nc.dram_tensor("xb", [G, R, pad], bf16, kind="ExternalInput").ap()
    wb = nc.dram_tensor("wb", [G, 128, WCOLS], bf16, kind="ExternalInput").ap()
    if has_bh:
        bb = nc.dram_tensor("bb", [G, 128, 2], f32, kind="ExternalInput").ap()
    o = nc.dram_tensor("o", [G, 4, pad], f32, kind="ExternalOutput").ap()

    tanh = mybir.ActivationFunctionType.Tanh

    with tile.TileContext(nc) as tc, ExitStack() as ctx:
        consts = ctx.enter_context(tc.tile_pool(name="consts", bufs=1))
        hpool = ctx.enter_context(tc.tile_pool(name="hs", bufs=3))
        opool = ctx.enter_context(tc.tile_pool(name="os", bufs=1))
        psum = ctx.enter_context(tc.tile_pool(name="ps", bufs=1, space="PSUM"))

        # <=8 input/output DMAs total stay within the 8 HWDGE queues, so no
        # DMA carries a queue-reuse wait on top of its data wait.
        xb_t, wb_t, wi_t, wh_t, bh_t, wo_t, o_sb = {}, {}, {}, {}, {}, {}, {}
        for g in range(G):
            xb_t[g] = consts.tile([R, pad], bf16, tag=f"xb{g}", name=f"xbt{g}")
            nc.sync.dma_start(out=xb_t[g][:], in_=xb[g])
            wb_t[g] = consts.tile([128, WCOLS], bf16, tag=f"wb{g}",
                                  name=f"wbt{g}")
            nc.sync.dma_start(out=wb_t[g][:], in_=wb[g])
            wi_t[g] = wb_t[g][0:R, 0:128]
            wh_t[g, 0] = wb_t[g][:, 128:256]
            wh_t[g, 1] = wb_t[g][:, 256:384]
            wo_t[g] = wb_t[g][:, 384:388]
            if has_bh:
                bbt = consts.tile([128, 2], f32, tag=f"bb{g}", name=f"bbt{g}")
                nc.sync.dma_start(out=bbt[:], in_=bb[g])
                bh_t[g, 0] = bbt[:, 0:1]
                bh_t[g, 1] = bbt[:, 1:2]
            else:
                bh_t[g, 0] = bh_t[g, 1] = None

        # Warm the PE_HAM clock gate (1.2 -> 2.4 GHz needs ~3.4 us of
        # sustained PE activity) while the input DMAs are still in flight:
        # fp32 matmuls at 4 cyc/row burn ~1.7us of PE busy each.
        warm = hpool.tile([128, T], f32, tag="warm", name="warm")
        nc.vector.memset(warm[:], 0.0)
        wp = psum.tile([1, T], f32, tag="po", bufs=2, name="wp",
                       padded_shape=[4, T])
        for i in range(2):
            nc.tensor.matmul(wp[:], warm[:, 0:1], warm[:],
                             start=True, stop=True, skip_group_check=True)
        # Pull the ~2.7us tanh ACT_TABLE_LOAD into the DMA window.
        dact = hpool.tile([1, 8], f32, tag="dact", name="dact")
        nc.scalar.activation(dact[:], warm[0:1, 0:8], tanh)

        # Throwaway matmuls absorb the preamble DMA semaphore waits into
        # the PE clock, so steady-state matmuls carry at most one wait.
        dp = psum.tile([1, 1], f32, tag="po", bufs=2, name="dp",
                       padded_shape=[4, T])
        for g in range(G):
            for i, wt in enumerate((wb_t[g], xb_t[g])):
                w1 = wt[:, 0:1]
                nc.tensor.matmul(dp[:], w1, w1, start=(g == 0 and i == 0),
                                 stop=(g == G - 1 and i == 1),
                                 skip_group_check=True)

        nblk = sum(len(_block_sizes(pad, g == G - 1)) for g in range(G))
        for g in range(G):
            o_sb[g] = opool.tile([4, pad], f32, tag=f"o{g}", name=f"osb{g}")
            sizes = _block_sizes(pad, g == G - 1)
            offs = [sum(sizes[:i]) for i in range(len(sizes))]
            for boff, bsz in zip(offs, sizes):
                p1 = psum.tile([128, bsz], f32, tag="p1", bufs=1,
                               padded_shape=[128, B])
                for moff, msz in _mm_splits(bsz):
                    nc.tensor.matmul(p1[:, moff:moff + msz], wi_t[g],
                                     xb_t[g][:, boff + moff:boff + moff + msz],
                                     start=True, stop=True)
                h1 = hpool.tile([128, bsz], bf16, tag="h1", bufs=nblk,
                                padded_shape=[128, B])
                nc.scalar.activation(h1[:], p1[:], tanh)
                p2 = psum.tile([128, bsz], f32, tag="p2", bufs=1,
                               padded_shape=[128, B])
                for moff, msz in _mm_splits(bsz):
                    nc.tensor.matmul(p2[:, moff:moff + msz], wh_t[g, 0],
                                     h1[:, moff:moff + msz],
                                     start=True, stop=True)
                h2 = hpool.tile([128, bsz], bf16, tag="h2", bufs=nblk,
                                padded_shape=[128, B])
                if has_bh:
                    nc.scalar.activation(h2[:], p2[:], tanh, bias=bh_t[g, 0])
                else:
                    nc.scalar.activation(h2[:], p2[:], tanh)
                p3 = psum.tile([128, bsz], f32, tag="p3", bufs=1,
                               padded_shape=[128, B])
                for moff, msz in _mm_splits(bsz):
                    nc.tensor.matmul(p3[:, moff:moff + msz], wh_t[g, 1],
                                     h2[:, moff:moff + msz],
                                     start=True, stop=True)
                h3 = hpool.tile([128, bsz], bf16, tag="h3", bufs=nblk,
                                padded_shape=[128, B])
                if has_bh:
                    nc.scalar.activation(h3[:], p3[:], tanh, bias=bh_t[g, 1])
                else:
                    nc.scalar.activation(h3[:], p3[:], tanh)
                for moff, msz in _mm_splits(bsz):
                    po = psum.tile([4, msz], f32, tag="po", bufs=2,
                                   padded_shape=[4, T])
                    nc.tensor.matmul(po[:], wo_t[g], h3[:, moff:moff + msz],
                                     start=True, stop=True)
                    c0 = boff + moff
                    nc.vector.tensor_copy(o_sb[g][:, c0:c0 + msz], po[:])
            nc.sync.dma_start(out=o[g], in_=o_sb[g][:])
    nc.compile()
    return nc


def _route(x, lo_core, hi_core, swin):
    """Per-subdomain point lists: s covers p iff all window sigmoid args >= -TAU."""
    n = x.shape[0]
    pts = []
    for si in range(S):
        m = np.ones(n, dtype=bool)
        for d in range(N_DIM):
            sd = swin[si, d]
            lo, hi = lo_core[si, d], hi_core[si, d]
            if sd >= 0:
                m &= (x[:, d] >= lo - TAU / max(sd, 1e-30)) \
                    & (x[:, d] <= hi + TAU / max(sd, 1e-30))
            else:  # pathological geometry; sigmoids flip direction
                m &= (x[:, d] <= lo + TAU / max(-sd, 1e-30)) \
                    & (x[:, d] >= hi - TAU / max(-sd, 1e-30))
        pts.append(np.nonzero(m)[0])
    return pts


def _pack(x, args64, pts, pad, center, half_w, has_bin, has_bh):
    """Build the per-core device input tensors (bf16 hi/lo packing)."""
    import ml_dtypes
    bf = ml_dtypes.bfloat16
    R = 18 if has_bin else 16
    in_maps = []
    for c in range(NCORES):
        xbv = np.zeros((G, R, pad), bf)
        wbv = np.zeros((G, 128, WCOLS), bf)
        bbv = np.zeros((G, 128, 2), np.float32)
        wi = wbv[:, 0:R, 0:128]
        wh0 = wbv[:, :, 128:256]
        wh1 = wbv[:, :, 256:384]
        wo = wbv[:, :, 384:388]
        for g in range(G):
            for j in range(4):
                s_ = c * SUB_PER_CORE + g * 4 + j
                idx = pts[s_]
                cnt = len(idx)
                xn = (x[idx].astype(np.float64) - center[s_]) / half_w[s_]
                xn_hi = xn.astype(bf)
                xn_lo = (xn - xn_hi.astype(np.float64)).astype(bf)
                for d in range(N_DIM):
                    xbv[g, 4 * j + 2 * d, :cnt] = xn_hi[:, d]
                    xbv[g, 4 * j + 2 * d + 1, :cnt] = xn_lo[:, d]
                r = slice(32 * j, 32 * j + 32)
                w_in = args64["W_in"][s_]          # (H, D)
                for d in range(N_DIM):
                    wi[g, 4 * j + 2 * d, r] = w_in[:, d].astype(bf)
                    wi[g, 4 * j + 2 * d + 1, r] = w_in[:, d].astype(bf)
                if has_bin:
                    xbv[g, 16, :] = 1.0
                    xbv[g, 17, :] = 1.0
                    b_hi = args64["b_in"][s_].astype(bf)
                    wi[g, 16, r] = b_hi
                    wi[g, 17, r] = (args64["b_in"][s_]
                                    - b_hi.astype(np.float64)).astype(bf)
                wh0[g, r, r] = args64["W_h1"][s_].T.astype(bf)
                wh1[g, r, r] = args64["W_h2"][s_].T.astype(bf)
                bbv[g, r, 0] = args64["b_h1"][s_]
                bbv[g, r, 1] = args64["b_h2"][s_]
                wo[g, r, j] = args64["W_out"][s_, 0].astype(bf)
        m = {"xb": xbv, "wb": wbv}
        if has_bh:
            m["bb"] = bbv
        in_maps.append(m)
    return in_maps


def _host_reference(x, lo_core, hi_core, lo_ext, hi_ext,
                    W_in, b_in, W_h1, b_h1, W_h2, b_h2, W_out, b_out):
    """Dense fallback (numpy, chunked) for inputs without FBPINN locality."""
    center = (lo_ext + hi_ext) * 0.5
    half_w = (hi_ext - lo_ext) * 0.5
    overlap = np.maximum(hi_ext - hi_core, lo_core - lo_ext)
    width = hi_ext - lo_ext
    s = 4.0 / (2.0 * overlap * width + 1e-8)
    sigm = lambda v: 1.0 / (1.0 + np.exp(-v))
    outs = []
    for i in range(0, x.shape[0], 8192):
        xc = x[i:i + 8192].astype(np.float64)
        xn = (xc[None] - center[:, None]) / half_w[:, None]
        hh = np.tanh(np.einsum("snd,shd->snh", xn, W_in) + b_in[:, None])
        hh = np.tanh(np.einsum("snh,skh->snk", hh, W_h1) + b_h1[:, None])
        hh = np.tanh(np.einsum("snh,skh->snk", hh, W_h2) + b_h2[:, None])
        out = np.einsum("snh,soh->sno", hh, W_out) + b_out[:, None]
        out = out * SCALE + SHIFT
        left = sigm(s[:, None] * (xc[None] - lo_core[:, None]))
        right = sigm(s[:, None] * (hi_core[:, None] - xc[None]))
        w = np.prod(left * right, axis=-1, keepdims=True)
        w = w / (np.sum(w, axis=0, keepdims=True) + 1e-8)
        u = np.sum(out * w, axis=0)
        gg = -np.sin(np.pi * xc[:, 1])[:, None]
        fac = (np.tanh(xc[:, 1] + 1) * np.tanh(xc[:, 1] - 1)
               * np.tanh(xc[:, 0]))[:, None]
        outs.append((gg + fac * u).astype(np.float32))
    return np.concatenate(outs, axis=0)


def _prepare(x, args64):
    """Routing + normalization geometry. Returns (pts, pad, swin, center,
    half_w) or None if the inputs lack FBPINN locality (dense fallback)."""
    lo_core64, hi_core64 = args64["lo_core"], args64["hi_core"]
    lo_ext64, hi_ext64 = args64["lo_ext"], args64["hi_ext"]
    n = x.shape[0]
    center = (lo_ext64 + hi_ext64) * 0.5
    half_w = (hi_ext64 - lo_ext64) * 0.5
    overlap = np.maximum(hi_ext64 - hi_core64, lo_core64 - lo_ext64)
    width = hi_ext64 - lo_ext64
    swin = 4.0 / (2.0 * overlap * width + 1e-8)

    pts = _route(x, lo_core64, hi_core64, swin)
    counts = np.array([len(p) for p in pts])
    if counts.sum() > 4 * n or counts.max() > max(4 * n // S, 8192):
        return None
    pad = int(max(128, -(-counts.max() // 128) * 128))
    return pts, pad, swin, center, half_w


def _epilogue(x, args64, pts, swin, o_by_sub):
    """Window weights + normalized scatter-add + boundary condition.
    o_by_sub: callable s -> raw device MLP outputs for subdomain s's slots."""
    n = x.shape[0]
    lo_core64, hi_core64 = args64["lo_core"], args64["hi_core"]
    b_out64 = args64["b_out"]
    numer = np.zeros(n, np.float64)
    denom = np.zeros(n, np.float64)
    sigm = lambda v: 1.0 / (1.0 + np.exp(-v))
    for s_ in range(S):
        idx = pts[s_]
        cnt = len(idx)
        if cnt == 0:
            continue
        xs = x[idx].astype(np.float64)
        arg_l = swin[s_] * (xs - lo_core64[s_])
        arg_r = swin[s_] * (hi_core64[s_] - xs)
        w = np.prod(sigm(arg_l) * sigm(arg_r), axis=-1)
        out_s = (o_by_sub(s_)[:cnt].astype(np.float64)
                 + b_out64[s_, 0]) * SCALE + SHIFT
        np.add.at(numer, idx, out_s * w)
        np.add.at(denom, idx, w)
    u = numer / (denom + 1e-8)
    x64 = x.astype(np.float64)
    gg = -np.sin(np.pi * x64[:, 1])
    fac = np.tanh(x64[:, 1] + 1.0) * np.tanh(x64[:, 1] - 1.0) * np.tanh(x64[:, 0])
    return (gg + fac * u)[:, None].astype(np.float32)


def kernel(x, lo_core, hi_core, lo_ext, hi_ext,
           W_in, b_in, W_h1, b_h1, W_h2, b_h2, W_out, b_out,
           _profile=False):
    x = np.asarray(x, np.float32)
    args64 = {k: np.asarray(v, np.float64) for k, v in dict(
        lo_core=lo_core, hi_core=hi_core, lo_ext=lo_ext, hi_ext=hi_ext,
        W_in=W_in, b_in=b_in, W_h1=W_h1, b_h1=b_h1, W_h2=W_h2, b_h2=b_h2,
        W_out=W_out, b_out=b_out).items()}

    prep = _prepare(x, args64)
    if prep is None:
        return _host_reference(x, **args64)
    pts, pad, swin, center, half_w = prep

    has_bin = bool(np.any(args64["b_in"] != 0.0))
    has_bh = bool(np.any(args64["b_h1"] != 0.0)
                  or np.any(args64["b_h2"] != 0.0))
    in_maps = _pack(x, args64, pts, pad, center, half_w, has_bin, has_bh)

    from concourse.bass_utils import run_bass_kernel_spmd
    key = (pad, has_bin, has_bh)
    if key not in _BUILD_CACHE:
        _BUILD_CACHE[key] = _build_bass(pad, has_bin, has_bh)
    nc = _BUILD_CACHE[key]
    res = run_bass_kernel_spmd(nc, in_maps, list(range(NCORES)),
                               trace=bool(_profile))

    def o_by_sub(s_):
        c, rem = divmod(s_, SUB_PER_CORE)
        g, j = divmod(rem, 4)
        return res.results[c]["o"][g, j]

    final = _epilogue(x, args64, pts, swin, o_by_sub)
    if _profile:
        return final, res
    return final
